# revision 11
# baseline (speedup 1.0000x reference)
"""MoE top-k router kernel for Trainium2 (8 NeuronCores, SPMD).

Computes, for hidden_states [4, 4096, 2048] f32 and gate_w [64, 2048] f32:
  router_logits    = hidden @ gate_w.T            [4, 4096, 64] f32
  routing_weights  = softmax(top8(router_logits)) [4, 4096, 8]  f32
  selected_experts = argtop8(router_logits)       [4, 4096, 8]  int32

Sharding: tokens (batch*seq = 16384) split evenly across 8 cores (2048
tokens/core); the small [64, 2048] gate weight is replicated. No
cross-core communication.

Per-core dataflow:
  - gate_w loaded once, PE-transposed to gwT tiles [128h, 64e] per k-tile.
  - per 128-token tile: DMA the [128, 2048] hidden slab (rows contiguous in
    HBM -> line-rate), PE-transpose each [128, 128] chunk to h-major
    (matmul contracts over H, which must sit on partitions), copy PSUM->SBUF
    (split across DVE/ACT), then 16 accumulating fp32 matmuls produce
    logits [128tok, 64e] in PSUM.
  - top-8 via the DVE MAX8 / FIND_INDEX8 hardware ops; softmax over the 8
    slots with ACT exp (+accumulated sum) and DVE reciprocal.
"""

import os
import numpy as np

import concourse.bass as bass
import concourse.bacc as bacc
import concourse.mybir as mybir
from concourse import tile
from concourse.bass_utils import run_bass_kernel_spmd
from concourse.masks import make_identity

B, S, H = 4, 4096, 2048
E, TOPK = 64, 8
NCORES = 8
T = B * S              # 16384 tokens total
TPC = T // NCORES      # 2048 tokens per core
P = 128
KT = H // P            # 16 contraction tiles
TT = TPC // P          # 16 token tiles per core

F32 = mybir.dt.float32
F32R = mybir.dt.float32r
U32 = mybir.dt.uint32

_DT = {"f32": F32, "f32r": F32R}


def _body(tc, x, w, lo, wo, eo, mm_dt, tr_dt, reps=1):
    nc = tc.nc

    SLAB = 4           # k-chunks per PSUM->SBUF copy batch
    NSLAB = KT // SLAB

    with (
        tc.tile_pool(name="consts", bufs=1) as consts,
        tc.tile_pool(name="xin", bufs=3) as xpool,
        tc.tile_pool(name="hT", bufs=6) as hpool,
        tc.tile_pool(name="small", bufs=4) as sm,
    ):
        ident = consts.tile([P, P], F32)
        make_identity(nc, ident)

        # gate_w [64, 2048] -> gwT [128h, kt, 64e]; issued on the ACT HWDGE
        # ring so it doesn't serialize behind the x stream on the SP ring.
        # Chunked DMA so the first transposes start ~0.8us in, and 4
        # transposes share one PSUM tile per copy to keep PE unblocked.
        gw_sb = consts.tile([E, H], F32)
        for c in range(4):
            nc.scalar.dma_start(
                gw_sb[:, c * (H // 4):(c + 1) * (H // 4)],
                w[:, c * (H // 4):(c + 1) * (H // 4)],
            )
        gwT = consts.tile([P, KT, E], F32)
        with tc.tile_pool(name="setup_ps", bufs=2, space="PSUM") as setup_ps:
            for c in range(4):
                tp = setup_ps.tile([P, 4, E], F32)
                for q in range(4):
                    k = c * 4 + q
                    nc.tensor.transpose(
                        tp[:, q, :], gw_sb[:, k * P:(k + 1) * P], ident[:E, :E]
                    )
                nc.scalar.copy(gwT[:, c * 4:(c + 1) * 4, :], tp)

        with (
            tc.tile_pool(name="tpsum", bufs=5, space="PSUM") as tpsum,
            tc.tile_pool(name="lgps", bufs=3, space="PSUM") as lgpsum,
        ):
            _main(tc, x, lo, wo, eo, ident, gwT,
                  xpool, hpool, sm, tpsum, lgpsum, reps)


def _main(tc, x, lo, wo, eo, ident, gwT, xpool, hpool, sm, tpsum, lgpsum, reps):
    nc = tc.nc
    SLAB = 4
    NSLAB = KT // SLAB
    GRP = 4            # token tiles staged per output DMA

    lo_r = lo.rearrange("(g tt p) e -> g p tt e", p=P, tt=GRP)
    wo_r = wo.rearrange("(g tt p) k -> g p tt k", p=P, tt=GRP)
    eo_r = eo.rearrange("(g tt p) k -> g p tt k", p=P, tt=GRP)

    for ti in range(TT * reps):
        t = ti % TT
        g, tt = t // GRP, t % GRP
        xt = xpool.tile([P, H], F32)
        half = H // 2
        nc.sync.dma_start(xt[:, :half], x[t * P:(t + 1) * P, :half])
        nc.sync.dma_start(xt[:, half:], x[t * P:(t + 1) * P, half:])

        lg_ps = lgpsum.tile([P, E], F32)
        for s in range(NSLAB):
            tp = tpsum.tile([P, SLAB * P], F32)
            for q in range(SLAB):
                k = s * SLAB + q
                nc.tensor.transpose(
                    tp[:, q * P:(q + 1) * P],
                    xt[:, k * P:(k + 1) * P],
                    ident,
                )
            hT = hpool.tile([P, SLAB * P], F32)
            if s % 2 == 0:
                nc.vector.tensor_copy(hT, tp)
            else:
                nc.scalar.copy(hT, tp)
            for q in range(SLAB):
                k = s * SLAB + q
                nc.tensor.matmul(
                    lg_ps,
                    lhsT=hT[:, q * P:(q + 1) * P],
                    rhs=gwT[:, k, :],
                    start=(k == 0),
                    stop=(k == KT - 1),
                )

        if tt == 0:
            lg_st = sm.tile([P, GRP, E], F32, tag="lg_st")
            wt_st = sm.tile([P, GRP, TOPK], F32, tag="wt_st")
            ix_st = sm.tile([P, GRP, TOPK], U32, tag="ix_st")
        lg_sb = lg_st[:, tt, :]
        nc.scalar.copy(lg_sb, lg_ps)

        vals = sm.tile([P, TOPK], F32)
        nc.vector.max(out=vals, in_=lg_sb)
        nc.vector.max_index(ix_st[:, tt, :], vals, lg_sb)

        nm = sm.tile([P, 1], F32)
        nc.vector.tensor_scalar_mul(nm, vals[:, 0:1], -1.0)
        ew = sm.tile([P, TOPK], F32)
        ssum = sm.tile([P, 1], F32)
        nc.scalar.activation(
            ew, vals, mybir.ActivationFunctionType.Exp,
            bias=nm, scale=1.0, accum_out=ssum,
        )
        rec = sm.tile([P, 1], F32)
        nc.vector.reciprocal(rec, ssum)
        nc.vector.tensor_scalar_mul(wt_st[:, tt, :], ew, rec)

        last_group = (t // GRP) == (TT // GRP) - 1
        if tt == GRP - 1 and not last_group:
            nc.scalar.dma_start(lo_r[g], lg_st)
            nc.scalar.dma_start(wo_r[g], wt_st)
            nc.scalar.dma_start(eo_r[g], ix_st)
        elif last_group:
            # final group: per-tile DMAs so the kernel tail doesn't wait
            # for the whole 4-tile stage after the last matmul
            nc.scalar.dma_start(lo_r[g, :, tt], lg_st[:, tt, :])
            nc.scalar.dma_start(wo_r[g, :, tt], wt_st[:, tt, :])
            nc.scalar.dma_start(eo_r[g, :, tt], ix_st[:, tt, :])


_programs = {}


def build_program(mm="f32", tr="f32", reps=1):
    key = (mm, tr, reps)
    if key in _programs:
        return _programs[key]
    nc = bacc.Bacc(
        "TRN2", target_bir_lowering=False, debug=False, enable_asserts=False
    )
    x = nc.dram_tensor("x", [TPC, H], F32, kind="ExternalInput").ap()
    w = nc.dram_tensor("w", [E, H], F32, kind="ExternalInput").ap()
    lo = nc.dram_tensor("logits", [TPC, E], F32, kind="ExternalOutput").ap()
    wo = nc.dram_tensor("weights", [TPC, TOPK], F32, kind="ExternalOutput").ap()
    eo = nc.dram_tensor("experts", [TPC, TOPK], U32, kind="ExternalOutput").ap()
    with tile.TileContext(nc) as tc:
        _body(tc, x, w, lo, wo, eo, _DT[mm], _DT[tr], reps=reps)
    nc.compile()
    _programs[key] = nc
    return nc


def _shard_inputs(hidden_states, gate_w):
    hs = np.ascontiguousarray(np.asarray(hidden_states, dtype=np.float32))
    gw = np.ascontiguousarray(np.asarray(gate_w, dtype=np.float32))
    shards = hs.reshape(NCORES, TPC, H)
    return [
        {"x": np.ascontiguousarray(shards[i]), "w": gw} for i in range(NCORES)
    ]


def _gather(results):
    logits = np.concatenate([r["logits"] for r in results], axis=0)
    weights = np.concatenate([r["weights"] for r in results], axis=0)
    experts = np.concatenate([r["experts"] for r in results], axis=0)
    return (
        logits.reshape(B, S, E).astype(np.float32),
        weights.reshape(B, S, TOPK).astype(np.float32),
        experts.astype(np.int32).reshape(B, S, TOPK),
    )


def run(hidden_states, gate_w, mm="f32", tr="f32", trace=False):
    nc = build_program(mm=mm, tr=tr)
    in_maps = _shard_inputs(hidden_states, gate_w)
    res = run_bass_kernel_spmd(
        nc, in_maps, core_ids=list(range(NCORES)), trace=trace
    )
    return _gather(res.results), res


def kernel(**inputs):
    mm = os.environ.get("ROUTER_MM_DT", "f32")
    tr = os.environ.get("ROUTER_TR_DT", "f32")
    (logits, weights, experts), _ = run(
        inputs["hidden_states"], inputs["gate_w"], mm=mm, tr=tr
    )
    return logits, weights, experts


# revision 15
# speedup vs baseline: 1.6330x; 1.6330x over previous
"""MoE top-k router kernel for Trainium2 (8 NeuronCores, SPMD).

Computes, for hidden_states [4, 4096, 2048] f32 and gate_w [64, 2048] f32:
  router_logits    = hidden @ gate_w.T            [4, 4096, 64] f32
  routing_weights  = softmax(top8(router_logits)) [4, 4096, 8]  f32
  selected_experts = argtop8(router_logits)       [4, 4096, 8]  int32

Sharding: tokens (batch*seq = 16384) split evenly across 8 cores (2048
tokens/core); the small [64, 2048] gate weight is replicated. No
cross-core communication.

Per-core dataflow:
  - gate_w loaded once, PE-transposed to gwT tiles [128h, 64e] per k-tile.
  - per 128-token tile: DMA the [128, 2048] hidden slab (rows contiguous in
    HBM -> line-rate), PE-transpose each [128, 128] chunk to h-major
    (matmul contracts over H, which must sit on partitions), copy PSUM->SBUF
    (split across DVE/ACT), then 16 accumulating fp32 matmuls produce
    logits [128tok, 64e] in PSUM.
  - top-8 via the DVE MAX8 / FIND_INDEX8 hardware ops; softmax over the 8
    slots with ACT exp (+accumulated sum) and DVE reciprocal.
"""

import os
import numpy as np

import concourse.bass as bass
import concourse.bacc as bacc
import concourse.mybir as mybir
from concourse import tile
from concourse.bass_utils import run_bass_kernel_spmd
from concourse.masks import make_identity

B, S, H = 4, 4096, 2048
E, TOPK = 64, 8
NCORES = 8
T = B * S              # 16384 tokens total
TPC = T // NCORES      # 2048 tokens per core
P = 128
KT = H // P            # 16 contraction tiles
TT = TPC // P          # 16 token tiles per core

F32 = mybir.dt.float32
F32R = mybir.dt.float32r
U32 = mybir.dt.uint32

_DT = {"f32": F32, "f32r": F32R}


def _body(tc, x, w, lo, wo, eo, mm_dt, tr_dt, reps=1):
    nc = tc.nc

    SLAB = 4           # k-chunks per PSUM->SBUF copy batch
    NSLAB = KT // SLAB

    with (
        tc.tile_pool(name="consts", bufs=1) as consts,
        tc.tile_pool(name="xin", bufs=4) as xpool,
        tc.tile_pool(name="hT", bufs=6) as hpool,
        tc.tile_pool(name="small", bufs=6) as sm,
    ):
        ident = consts.tile([P, P], F32)
        make_identity(nc, ident)

        # gate_w [64, 2048] -> gwT [128h, kt, 64e]; issued on the ACT HWDGE
        # ring so it doesn't serialize behind the x stream on the SP ring.
        # Chunked DMA so the first transposes start ~0.8us in, and 4
        # transposes share one PSUM tile per copy to keep PE unblocked.
        gw_sb = consts.tile([E, H], F32)
        for c in range(4):
            nc.scalar.dma_start(
                gw_sb[:, c * (H // 4):(c + 1) * (H // 4)],
                w[:, c * (H // 4):(c + 1) * (H // 4)],
            )
        gwT = consts.tile([P, KT, E], F32)
        with tc.tile_pool(name="setup_ps", bufs=2, space="PSUM") as setup_ps:
            for c in range(4):
                tp = setup_ps.tile([P, 4, E], F32)
                for q in range(4):
                    k = c * 4 + q
                    nc.tensor.transpose(
                        tp[:, q, :], gw_sb[:, k * P:(k + 1) * P], ident[:E, :E]
                    )
                nc.scalar.copy(gwT[:, c * 4:(c + 1) * 4, :], tp)

        with (
            tc.tile_pool(name="tpsum", bufs=3, space="PSUM") as tpsum,
            tc.tile_pool(name="lgps", bufs=2, space="PSUM") as lgpsum,
        ):
            _main(tc, x, lo, wo, eo, ident, gwT,
                  xpool, hpool, sm, tpsum, lgpsum, reps)


def _main(tc, x, lo, wo, eo, ident, gwT, xpool, hpool, sm, tpsum, lgpsum, reps):
    nc = tc.nc
    SLAB = 8
    NSLAB = KT // SLAB
    GRP = 4            # token tiles staged per output DMA

    lo_r = lo.rearrange("(g tt p) e -> g p tt e", p=P, tt=GRP)
    wo_r = wo.rearrange("(g tt p) k -> g p tt k", p=P, tt=GRP)
    eo_r = eo.rearrange("(g tt p) k -> g p tt k", p=P, tt=GRP)

    for ti in range(TT * reps):
        t = ti % TT
        g, tt = t // GRP, t % GRP
        xt = xpool.tile([P, H], F32)
        half = H // 2
        nc.sync.dma_start(xt[:, :half], x[t * P:(t + 1) * P, :half])
        nc.sync.dma_start(xt[:, half:], x[t * P:(t + 1) * P, half:])

        lg_ps = lgpsum.tile([P, E], F32)
        for s in range(NSLAB):
            tp = tpsum.tile([P, SLAB * P], F32)
            for q in range(SLAB):
                k = s * SLAB + q
                nc.tensor.transpose(
                    tp[:, q * P:(q + 1) * P],
                    xt[:, k * P:(k + 1) * P],
                    ident,
                )
            hT = hpool.tile([P, SLAB * P], F32)
            if s % 2 == 0:
                nc.vector.tensor_copy(hT, tp)
            else:
                nc.scalar.copy(hT, tp)
            for q in range(SLAB):
                k = s * SLAB + q
                nc.tensor.matmul(
                    lg_ps,
                    lhsT=hT[:, q * P:(q + 1) * P],
                    rhs=gwT[:, k, :],
                    start=(k == 0),
                    stop=(k == KT - 1),
                )

        if tt == 0:
            lg_st = sm.tile([P, GRP, E], F32, tag="lg_st")
            wt_st = sm.tile([P, GRP, TOPK], F32, tag="wt_st")
            ix_st = sm.tile([P, GRP, TOPK], U32, tag="ix_st")
        lg_sb = lg_st[:, tt, :]
        nc.scalar.copy(lg_sb, lg_ps)

        # final group: per-tile DMAs, each issued as soon as its data is
        # ready, so the kernel tail after the last matmul stays short
        last_group = (t // GRP) == (TT // GRP) - 1
        if last_group:
            nc.scalar.dma_start(lo_r[g, :, tt], lg_st[:, tt, :])

        vals = sm.tile([P, TOPK], F32)
        nc.vector.max(out=vals, in_=lg_sb)
        nc.vector.max_index(ix_st[:, tt, :], vals, lg_sb)
        if last_group:
            nc.scalar.dma_start(eo_r[g, :, tt], ix_st[:, tt, :])

        nm = sm.tile([P, 1], F32)
        nc.vector.tensor_scalar_mul(nm, vals[:, 0:1], -1.0)
        ew = sm.tile([P, TOPK], F32)
        ssum = sm.tile([P, 1], F32)
        nc.scalar.activation(
            ew, vals, mybir.ActivationFunctionType.Exp,
            bias=nm, scale=1.0, accum_out=ssum,
        )
        rec = sm.tile([P, 1], F32)
        nc.vector.reciprocal(rec, ssum)
        nc.vector.tensor_scalar_mul(wt_st[:, tt, :], ew, rec)

        if last_group:
            nc.scalar.dma_start(wo_r[g, :, tt], wt_st[:, tt, :])
        elif tt == GRP - 1:
            nc.scalar.dma_start(lo_r[g], lg_st)
            nc.scalar.dma_start(wo_r[g], wt_st)
            nc.scalar.dma_start(eo_r[g], ix_st)


_programs = {}


def build_program(mm="f32", tr="f32", reps=1):
    key = (mm, tr, reps)
    if key in _programs:
        return _programs[key]
    nc = bacc.Bacc(
        "TRN2", target_bir_lowering=False, debug=False, enable_asserts=False
    )
    x = nc.dram_tensor("x", [TPC, H], F32, kind="ExternalInput").ap()
    w = nc.dram_tensor("w", [E, H], F32, kind="ExternalInput").ap()
    lo = nc.dram_tensor("logits", [TPC, E], F32, kind="ExternalOutput").ap()
    wo = nc.dram_tensor("weights", [TPC, TOPK], F32, kind="ExternalOutput").ap()
    eo = nc.dram_tensor("experts", [TPC, TOPK], U32, kind="ExternalOutput").ap()
    with tile.TileContext(nc) as tc:
        _body(tc, x, w, lo, wo, eo, _DT[mm], _DT[tr], reps=reps)
    nc.compile()
    _programs[key] = nc
    return nc


def _shard_inputs(hidden_states, gate_w):
    hs = np.ascontiguousarray(np.asarray(hidden_states, dtype=np.float32))
    gw = np.ascontiguousarray(np.asarray(gate_w, dtype=np.float32))
    shards = hs.reshape(NCORES, TPC, H)
    return [
        {"x": np.ascontiguousarray(shards[i]), "w": gw} for i in range(NCORES)
    ]


def _gather(results):
    logits = np.concatenate([r["logits"] for r in results], axis=0)
    weights = np.concatenate([r["weights"] for r in results], axis=0)
    experts = np.concatenate([r["experts"] for r in results], axis=0)
    return (
        logits.reshape(B, S, E).astype(np.float32),
        weights.reshape(B, S, TOPK).astype(np.float32),
        experts.astype(np.int32).reshape(B, S, TOPK),
    )


def run(hidden_states, gate_w, mm="f32", tr="f32", trace=False):
    nc = build_program(mm=mm, tr=tr)
    in_maps = _shard_inputs(hidden_states, gate_w)
    res = run_bass_kernel_spmd(
        nc, in_maps, core_ids=list(range(NCORES)), trace=trace
    )
    return _gather(res.results), res


def kernel(**inputs):
    mm = os.environ.get("ROUTER_MM_DT", "f32")
    tr = os.environ.get("ROUTER_TR_DT", "f32")
    (logits, weights, experts), _ = run(
        inputs["hidden_states"], inputs["gate_w"], mm=mm, tr=tr
    )
    return logits, weights, experts
